# revision 41
# baseline (speedup 1.0000x reference)
"""GCN 2-layer encoder on 8 Trainium2 NeuronCores (Bass/Tile).

kernel(**inputs) takes the FULL inputs and returns the FULL [80000, 32] f32
output.  Strategy (node partition across 8 cores, per sharding hint), ONE
fused SPMD launch with chunked in-kernel AllGathers:

  gcn_conv(x, W, b) = b + dinv * (A_hat @ (dinv * (x @ W)))  with self-loops,
  where dinv = 1/sqrt(indeg+1) and A_hat is the (unnormalized) adjacency.

  Phase A: z1 = dinv * (x @ W1)         (each core, its 10112-row node shard)
  AllGather(z1) in 3 chunks of 27/27/25 tiles -> chunk k's collective fires
           as soon as phase A finishes those tiles; each chunk is one int16
           gather range (<= 27648 rows), so phase B's range-k gathers start
           as soon as chunk k lands.
  Phase B: per dst-node-tile (128 nodes) gather z1 rows by edge source
           (gpsimd dma_gather, 256B rows, calls grouped over 4 dst tiles)
           and reduce with a one-hot scatter-matmul on the PE into PSUM;
           epilogue z2 = relu(dinv*agg + b1) * dinv (written rank-ordered).
  AllGather(z2) in 3 chunks (same overlap, both sides).
  Phase C: same aggregation over z2; epilogue out = (dinv*agg) @ W2 + b2.

  Edges are grouped by (dst-tile group, src chunk, dst tile) with counts
  padded to a global per-rank max so all 8 cores run one identical SPMD
  program; int16 gather indices are relative to the per-chunk AllGather
  output and sorted by source address within each group for HBM locality.
"""
import sys

sys.path.insert(0, '/opt/trn_rl_repo')

import numpy as np
import jax
from jax.sharding import Mesh, PartitionSpec
from jax.experimental.shard_map import shard_map

import concourse.bass as bass
import concourse.bacc as bacc
import concourse.tile as tile
import concourse.mybir as mybir
from concourse import bass2jax
from concourse.bass2jax import _bass_exec_p, partition_id_tensor
from concourse.masks import make_identity

F32 = mybir.dt.float32
BF16 = mybir.dt.bfloat16
I16 = mybir.dt.int16

N_NODES = 80000
IN_CH = 128
HID = 64
OUT_CH = 32
N_CORES = 8
NT = N_NODES // 128                     # 625 dst tiles
TPC = (NT + N_CORES - 1) // N_CORES     # 79 tile ranks per core
ROWS = TPC * 128                        # 10112 rows per core
CH_TILES = [27, 27, 25]                 # AllGather chunking (tiles per chunk)
CH_START = [0, 27, 54]
RK = [t * 128 for t in CH_TILES]        # rows per core per chunk
R = 3                                   # one gather range per chunk
G = 4                                   # dst tiles per gather-call group
NG = (TPC + G - 1) // G
CALL_MAX = 896                          # max indices per dma_gather call
SB = 8                                  # one-hot S matrices built per batch


def _ceil128(x):
    return ((x + 127) // 128) * 128


def _layer_pre(r_id, ridx, d_all, core_of_tile, tile_of, rank_of_tile):
    """Per-layer gather/scatter metadata.

    r_id:  per-edge source chunk (0..2)
    ridx:  per-edge gather index within that chunk's AllGather output
    """
    tile_g = d_all >> 7
    cnt_tile_r = np.bincount(tile_g * R + r_id,
                             minlength=NT * R).reshape(NT, R)
    cntK = np.zeros((N_CORES, TPC, R), np.int64)
    for c in range(N_CORES):
        real = tile_of[c][tile_of[c] >= 0]
        cntK[c, :len(real)] = cnt_tile_r[real]
    K = _ceil128(cntK.max(axis=0))      # [TPC, R] padded slot counts

    # segment order: (group, range, tile-in-group)
    seg_order = []
    for g in range(NG):
        ts = list(range(g * G, min((g + 1) * G, TPC)))
        for r in range(R):
            for t in ts:
                seg_order.append(t * R + r)
    seg_order = np.asarray(seg_order)
    seg_pos = np.empty(TPC * R, np.int64)
    seg_pos[seg_order] = np.arange(TPC * R)
    Kflat = K.reshape(-1)
    Koff_ord = np.zeros(TPC * R + 1, np.int64)
    Koff_ord[1:] = np.cumsum(Kflat[seg_order])
    T_pad = int(Koff_ord[-1])
    C_total = T_pad // 128
    seg_off = np.empty(TPC * R, np.int64)
    seg_off[seg_order] = Koff_ord[:-1]

    # pad slots gather *scattered* rows (same-row reads serialize on one
    # HBM bank); dstl=-1 keeps them out of the one-hot reduction
    rngpad = np.random.default_rng(12345)
    sizes = np.asarray([N_CORES * r for r in RK])
    gidx_all = np.empty((N_CORES, T_pad), np.int16)
    for r in range(R):
        for g in range(NG):
            ts = list(range(g * G, min((g + 1) * G, TPC)))
            lo = int(seg_off[ts[0] * R + r])
            hi = lo + int(sum(K[t, r] for t in ts))
            gidx_all[:, lo:hi] = rngpad.integers(
                0, sizes[r], (N_CORES, hi - lo)).astype(np.int16)
    dstl_all = np.full((N_CORES, T_pad), -1.0, np.float32)
    ecore = core_of_tile[tile_g]
    for c in range(N_CORES):
        m = ecore == c
        e_rank = rank_of_tile[tile_g[m]]
        e_r = r_id[m]
        seg = e_rank * R + e_r
        sp = ridx[m]
        order = np.lexsort((sp, seg_pos[seg]))
        seg_s = seg[order]                      # sorted by segment ordinal
        seg_counts = np.bincount(seg_s, minlength=TPC * R)
        starts = np.zeros(TPC * R, np.int64)    # seg id -> stream start
        csum = np.cumsum(seg_counts[seg_order])
        starts[seg_order[1:]] = csum[:-1]
        within = np.arange(len(seg_s)) - starts[seg_s]
        pos = seg_off[seg_s] + within
        gidx_all[c, pos] = sp[order].astype(np.int16)
        dstl_all[c, pos] = (d_all[m][order] & 127).astype(np.float32)

    # gather calls per (group, range), split at CALL_MAX; each call carries
    # the dst-tile rank of every 128-slot chunk it contains
    calls = []
    chunk_tiles = []                    # global chunk -> tile rank
    for g in range(NG):
        ts = list(range(g * G, min((g + 1) * G, TPC)))
        for r in range(R):
            slot_tiles = []
            for t in ts:
                slot_tiles += [t] * (int(K[t, r]) // 128)
            off = int(seg_off[ts[0] * R + r])
            while slot_tiles:
                take = min(len(slot_tiles), CALL_MAX // 128)
                calls.append((r, off, take * 128, slot_tiles[:take]))
                chunk_tiles += slot_tiles[:take]
                off += take * 128
                slot_tiles = slot_tiles[take:]
    first = {}
    last = {}
    for ci, t in enumerate(chunk_tiles):
        first.setdefault(t, ci)
        last[t] = ci

    idxw_all = np.zeros((N_CORES, 128, T_pad // 16), np.int16)
    for c in range(N_CORES):
        blk = gidx_all[c].reshape(T_pad // 16, 16).T
        idxw_all[c] = np.tile(blk, (8, 1))
    dstv_all = dstl_all.reshape(N_CORES, C_total, 128).transpose(0, 2, 1).copy()

    return dict(K=K, calls=calls, first=first, last=last, T_pad=T_pad,
                C_total=C_total, idxw=idxw_all, dstv=dstv_all)


def _preprocess(edge_index):
    src = np.asarray(edge_index[0], np.int64)
    dst = np.asarray(edge_index[1], np.int64)
    deg = np.bincount(dst, minlength=N_NODES).astype(np.float64) + 1.0
    dinv = (1.0 / np.sqrt(deg)).astype(np.float32)
    loop = np.arange(N_NODES, dtype=np.int64)
    s_all = np.concatenate([src, loop])
    d_all = np.concatenate([dst, loop])
    tile_g = d_all >> 7

    core_of_tile = np.minimum(np.arange(NT) // TPC, N_CORES - 1)
    tot_tile = np.bincount(tile_g, minlength=NT)
    tile_of = -np.ones((N_CORES, TPC), np.int64)
    for c in range(N_CORES):
        tl = np.where(core_of_tile == c)[0]
        order = tl[np.argsort(-tot_tile[tl], kind='stable')]
        tile_of[c, :len(order)] = order
    rank_of_tile = np.zeros(NT, np.int64)
    for c in range(N_CORES):
        real = tile_of[c][tile_of[c] >= 0]
        rank_of_tile[real] = np.arange(len(real))

    chunk_of = np.repeat(np.arange(R), CH_TILES)    # local tile idx -> chunk

    # layer 1: source position in the chunked AllGather of natural-order z1
    c1 = s_all // ROWS
    q1 = s_all - c1 * ROWS
    tl1 = q1 >> 7
    k1 = chunk_of[tl1]
    idx1 = c1 * np.asarray(RK)[k1] + (q1 - 128 * np.asarray(CH_START)[k1])

    # layer 2: source position in the chunked AllGather of rank-ordered z2
    t2 = s_all >> 7
    c2 = core_of_tile[t2]
    rk2 = rank_of_tile[t2]
    k2 = chunk_of[rk2]
    idx2 = (c2 * np.asarray(RK)[k2] + (rk2 - np.asarray(CH_START)[k2]) * 128
            + (s_all & 127))

    L1 = _layer_pre(k1, idx1, d_all, core_of_tile, tile_of, rank_of_tile)
    L2 = _layer_pre(k2, idx2, d_all, core_of_tile, tile_of, rank_of_tile)

    dinvS = np.ones((N_CORES, 128, TPC), np.float32)
    for c in range(N_CORES):
        for t in range(TPC):
            tl = tile_of[c, t]
            if tl >= 0:
                dinvS[c, :, t] = dinv[tl * 128:(tl + 1) * 128]
    dinvA = np.ones((N_CORES, 128, TPC), np.float32)
    for c in range(N_CORES):
        lo = c * ROWS
        hi = min((c + 1) * ROWS, N_NODES)
        dinvA[c, :, :(hi - lo) // 128] = dinv[lo:hi].reshape(-1, 128).T

    return dict(dinv=dinv, tile_of=tile_of, L1=L1, L2=L2,
                dinvS=dinvS, dinvA=dinvA)


def _emit_agg(nc, tc, src_aps, idx_sb, dstv_sb, dinv_sb, bb_sb, iota8,
              L, layer, out_d, t1_all=None):
    """Emit one aggregation phase: gather bf16 rows of src_aps[r] per edge
    (128-col elements, only the first HID columns carry data), scatter-
    matmul per dst tile into PSUM at full bf16 PE rate, apply the layer
    epilogue, write tile t to out_d[t*128:(t+1)*128, :]."""
    calls, first, last, C_total = L["calls"], L["first"], L["last"], \
        L["C_total"]
    tg = f"l{layer}"

    def epilogue(t, psum_t, ep, ps2):
        if layer == 1:
            t1 = ep.tile([128, HID], F32, tag=f"t1{tg}")
            nc.vector.tensor_scalar(out=t1[:], in0=psum_t[:],
                                    scalar1=dinv_sb[:, t:t + 1],
                                    scalar2=None,
                                    op0=mybir.AluOpType.mult)
            t2 = ep.tile([128, HID], F32, tag=f"t2{tg}")
            nc.vector.tensor_tensor(out=t2[:], in0=t1[:], in1=bb_sb[:],
                                    op=mybir.AluOpType.add)
            z2 = ep.tile([128, HID], F32, tag=f"z2{tg}")
            nc.scalar.activation(out=z2[:], in_=t2[:],
                                 func=mybir.ActivationFunctionType.Relu,
                                 scale=dinv_sb[:, t:t + 1])
            nc.sync.dma_start(out=out_d[t * 128:(t + 1) * 128, :], in_=z2[:])
        else:
            nc.vector.tensor_scalar(out=t1_all[:, t, :], in0=psum_t[:],
                                    scalar1=dinv_sb[:, t:t + 1],
                                    scalar2=None,
                                    op0=mybir.AluOpType.mult)

    with (
        tc.tile_pool(name=f"msgs{layer}", bufs=8) as mp,
        tc.tile_pool(name=f"s8_{layer}", bufs=4) as sp,
        tc.tile_pool(name=f"ep{layer}", bufs=4) as ep,
        tc.tile_pool(name=f"ps{layer}", bufs=8 if layer == 1 else 6,
                     space="PSUM") as ps,
    ):
        ps2 = None
        qn = 0
        ci = 0
        S8 = None
        acc = {}
        skip_mm = globals().get("_SKIP_MM", False)
        for (r, off, sz, slot_tiles) in calls:
            m = mp.tile([128, CALL_MAX // 128, HID], F32, tag=f"msgs{tg}")
            nc.gpsimd.dma_gather(
                out_ap=m[:, :sz // 128, :],
                in_ap=src_aps[r],
                idxs_ap=idx_sb[:, off // 16:(off + sz) // 16],
                num_idxs=sz,
                num_idxs_reg=sz,
                elem_size=HID,
                single_packet=True,
                queue_num=qn % 4,
            )
            qn += 1
            for s, t in enumerate(slot_tiles):
                if ci % SB == 0:
                    g0 = ci
                    gw = min(SB, C_total - g0)
                    S8 = sp.tile([128, SB, 128], F32, tag=f"s8{tg}")
                    dv = dstv_sb[:, g0:g0 + gw, None]\
                        .to_broadcast([128, gw, 128])
                    nc.vector.tensor_tensor(
                        out=S8[:, :gw, :], in0=iota8[:, :gw, :],
                        in1=dv, op=mybir.AluOpType.is_equal)
                if t not in acc:
                    acc[t] = ps.tile([128, HID], F32, space="PSUM",
                                     name=f"acc{tg}_{t}", tag=f"acc{tg}")
                if not skip_mm or ci == first[t] or ci == last[t]:
                    nc.tensor.matmul(out=acc[t][:, :],
                                     lhsT=S8[:, ci % SB, :],
                                     rhs=m[:, s, :],
                                     start=(ci == first[t]),
                                     stop=(ci == last[t]))
                if ci == last[t]:
                    epilogue(t, acc.pop(t), ep, ps2)
                ci += 1


def _build_merged(pre, prefix=5):
    L1, L2 = pre["L1"], pre["L2"]
    nc = bacc.Bacc("TRN2", target_bir_lowering=False, debug=False,
                   num_devices=N_CORES, num_swdge_queues=4,
                   dynamic_dma_scratch_size=16384)
    x_d = nc.dram_tensor("xT", [IN_CH, ROWS], F32, kind="ExternalInput")
    w1_d = nc.dram_tensor("w1", [IN_CH, HID], F32, kind="ExternalInput")
    w2_d = nc.dram_tensor("w2", [HID, OUT_CH], F32, kind="ExternalInput")
    dinvA_d = nc.dram_tensor("dinvA", [128, TPC], F32, kind="ExternalInput")
    dinvS_d = nc.dram_tensor("dinvS", [128, TPC], F32, kind="ExternalInput")
    bb1_d = nc.dram_tensor("bb1", [128, HID], F32, kind="ExternalInput")
    bb2_d = nc.dram_tensor("bb2", [128, OUT_CH], F32, kind="ExternalInput")
    idx1_d = nc.dram_tensor("idxw1", [128, L1["T_pad"] // 16], I16,
                            kind="ExternalInput")
    dstv1_d = nc.dram_tensor("dstv1", [128, L1["C_total"]], F32,
                             kind="ExternalInput")
    idx2_d = nc.dram_tensor("idxw2", [128, L2["T_pad"] // 16], I16,
                            kind="ExternalInput")
    dstv2_d = nc.dram_tensor("dstv2", [128, L2["C_total"]], F32,
                             kind="ExternalInput")
    out_d = nc.dram_tensor("outp", [ROWS, OUT_CH], F32, kind="ExternalOutput")

    with tile.TileContext(nc) as tc:
        with (
            tc.tile_pool(name="const", bufs=1) as cp,
            tc.tile_pool(name="dram", bufs=1, space="DRAM") as dram,
        ):
            ag1_in = dram.tile([ROWS, HID], F32)
            ag2_in = dram.tile([ROWS, HID], F32)
            _aspace = "Local" if globals().get("_NO_CC", False) else "Shared"
            ag1_out = [dram.tile([N_CORES * RK[k], HID], F32,
                                 addr_space=_aspace, name=f"ag1o{k}")
                       for k in range(R)]
            ag2_out = [dram.tile([N_CORES * RK[k], HID], F32,
                                 addr_space=_aspace, name=f"ag2o{k}")
                       for k in range(R)]

            ident = cp.tile([128, 128], F32)
            make_identity(nc, ident[:])
            w1sb = cp.tile([IN_CH, HID], F32)
            nc.sync.dma_start(out=w1sb[:], in_=w1_d.ap()[:, :])
            w2sb = cp.tile([HID, OUT_CH], F32)
            nc.sync.dma_start(out=w2sb[:], in_=w2_d.ap()[:, :])
            dinvA_sb = cp.tile([128, TPC], F32)
            nc.sync.dma_start(out=dinvA_sb[:], in_=dinvA_d.ap()[:, :])
            dinvS_sb = cp.tile([128, TPC], F32)
            nc.sync.dma_start(out=dinvS_sb[:], in_=dinvS_d.ap()[:, :])
            bb1_sb = cp.tile([128, HID], F32)
            nc.sync.dma_start(out=bb1_sb[:], in_=bb1_d.ap()[:, :])
            bb2_sb = cp.tile([128, OUT_CH], F32)
            nc.sync.dma_start(out=bb2_sb[:], in_=bb2_d.ap()[:, :])
            idx1_sb = cp.tile([128, L1["T_pad"] // 16], I16)
            nc.sync.dma_start(out=idx1_sb[:], in_=idx1_d.ap()[:, :])
            dstv1_sb = cp.tile([128, L1["C_total"]], F32)
            nc.sync.dma_start(out=dstv1_sb[:], in_=dstv1_d.ap()[:, :])
            idx2_sb = cp.tile([128, L2["T_pad"] // 16], I16)
            nc.sync.dma_start(out=idx2_sb[:], in_=idx2_d.ap()[:, :])
            dstv2_sb = cp.tile([128, L2["C_total"]], F32)
            nc.sync.dma_start(out=dstv2_sb[:], in_=dstv2_d.ap()[:, :])
            iota_i = cp.tile([128, SB * 128], I16)
            nc.gpsimd.iota(iota_i[:], pattern=[[0, SB], [1, 128]], base=0,
                           channel_multiplier=0)
            iota8 = cp.tile([128, SB, 128], F32)
            nc.vector.tensor_copy(out=iota8[:],
                                  in_=iota_i[:].rearrange("p (c f) -> p c f",
                                                          c=SB))
            t1_all = cp.tile([128, TPC, HID], F32)

            # ---- phase A: z1 = dinvA * (x @ W1) -> ag1_in
            # x arrives pre-transposed from the host: lhsT slices directly.
            with (
                tc.tile_pool(name="xt", bufs=1) as xtp,
                tc.tile_pool(name="zs", bufs=8) as zp,
                tc.tile_pool(name="psA", bufs=8, space="PSUM") as psA,
            ):
                xT_sb = xtp.tile([IN_CH, ROWS], F32)
                for k0 in range(0, TPC, 10):
                    k1 = min(k0 + 10, TPC)
                    nc.sync.dma_start(
                        out=xT_sb[:, k0 * 128:k1 * 128],
                        in_=x_d.ap()[:, k0 * 128:k1 * 128])
                for t in range(TPC):
                    zps = psA.tile([128, HID], F32, space="PSUM")
                    nc.tensor.matmul(out=zps[:],
                                     lhsT=xT_sb[:, t * 128:(t + 1) * 128],
                                     rhs=w1sb[:], start=True, stop=True)
                    zsb = zp.tile([128, HID], F32)
                    nc.vector.tensor_scalar(out=zsb[:], in0=zps[:],
                                            scalar1=dinvA_sb[:, t:t + 1],
                                            scalar2=None,
                                            op0=mybir.AluOpType.mult)
                    nc.sync.dma_start(out=ag1_in[t * 128:(t + 1) * 128, :],
                                      in_=zsb[:])

            no_cc = globals().get("_NO_CC", False)
            if prefix >= 2:
                for k in range(R):
                    lo = 128 * CH_START[k]
                    if no_cc:
                        for cc in range(N_CORES):
                            nc.sync.dma_start(
                                out=ag1_out[k][cc * RK[k]:(cc + 1) * RK[k], :],
                                in_=ag1_in[lo:lo + RK[k], :])
                    else:
                        nc.gpsimd.collective_compute(
                            "AllGather", mybir.AluOpType.bypass,
                            replica_groups=[list(range(N_CORES))],
                            ins=[ag1_in[lo:lo + RK[k], :]],
                            outs=[ag1_out[k][:]])

            if prefix >= 3:
                _emit_agg(nc, tc, [a[:] for a in ag1_out], idx1_sb, dstv1_sb,
                          dinvS_sb, bb1_sb, iota8, L1, layer=1,
                          out_d=ag2_in)

            if prefix >= 4:
                for k in range(R):
                    lo = 128 * CH_START[k]
                    if no_cc:
                        for cc in range(N_CORES):
                            nc.sync.dma_start(
                                out=ag2_out[k][cc * RK[k]:(cc + 1) * RK[k], :],
                                in_=ag2_in[lo:lo + RK[k], :])
                    else:
                        nc.gpsimd.collective_compute(
                            "AllGather", mybir.AluOpType.bypass,
                            replica_groups=[list(range(N_CORES))],
                            ins=[ag2_in[lo:lo + RK[k], :]],
                            outs=[ag2_out[k][:]])

            if prefix >= 5:
                _emit_agg(nc, tc, [a[:] for a in ag2_out], idx2_sb, dstv2_sb,
                          dinvS_sb, bb2_sb, iota8, L2, layer=2,
                          out_d=out_d.ap(), t1_all=t1_all)

                # software-pipelined final transform: out = t1 @ W2 + b2
                LAG = 4
                with (
                    tc.tile_pool(name="t1c", bufs=6) as tp1,
                    tc.tile_pool(name="tTs", bufs=6) as ttp,
                    tc.tile_pool(name="ob", bufs=4) as obp,
                    tc.tile_pool(name="psT", bufs=6, space="PSUM") as psT,
                    tc.tile_pool(name="psO", bufs=2, space="PSUM") as psO,
                ):
                    tT_s = [None] * TPC
                    for t in range(TPC + LAG):
                        if t < TPC:
                            t1c = tp1.tile([128, HID], F32, tag="t1c")
                            nc.vector.tensor_copy(out=t1c[:],
                                                  in_=t1_all[:, t, :])
                            tps = psT.tile([HID, 128], F32, space="PSUM",
                                           tag="tps")
                            nc.tensor.transpose(out=tps[:], in_=t1c[:],
                                                identity=ident[:])
                            tTt = ttp.tile([HID, 128], F32, tag="tT")
                            nc.vector.tensor_copy(out=tTt[:], in_=tps[:])
                            tT_s[t] = tTt
                        if t >= LAG:
                            u = t - LAG
                            op = psO.tile([128, OUT_CH], F32, space="PSUM",
                                          tag="op")
                            nc.tensor.matmul(out=op[:], lhsT=tT_s[u][:],
                                             rhs=w2sb[:], start=True,
                                             stop=True)
                            o = obp.tile([128, OUT_CH], F32, tag="o")
                            nc.vector.tensor_tensor(out=o[:], in0=op[:],
                                                    in1=bb2_sb[:],
                                                    op=mybir.AluOpType.add)
                            nc.sync.dma_start(
                                out=out_d.ap()[u * 128:(u + 1) * 128, :],
                                in_=o[:])
    nc.compile()
    return nc


class _SpmdRunner:
    def __init__(self, nc, n_cores=N_CORES):
        bass2jax.install_neuronx_cc_hook()
        self.nc = nc
        self.n_cores = n_cores
        in_names, out_names, out_avals = [], [], []
        partition_name = nc.partition_id_tensor.name if nc.partition_id_tensor \
            else None
        for alloc in nc.m.functions[0].allocations:
            if not isinstance(alloc, mybir.MemoryLocationSet):
                continue
            name = alloc.memorylocations[0].name
            if alloc.kind == "ExternalInput":
                if name != partition_name:
                    in_names.append(name)
            elif alloc.kind == "ExternalOutput":
                out_names.append(name)
                out_avals.append(jax.core.ShapedArray(
                    tuple(alloc.tensor_shape), mybir.dt.np(alloc.dtype)))
        self.in_names, self.out_names, self.out_avals = \
            in_names, out_names, out_avals
        n_params = len(in_names)
        n_outs = len(out_avals)
        all_names = list(in_names) + list(out_names)
        if partition_name is not None:
            all_names.append(partition_name)

        def _body(*args):
            operands = list(args)
            if partition_name is not None:
                operands.append(partition_id_tensor())
            outs = _bass_exec_p.bind(
                *operands,
                out_avals=tuple(out_avals),
                in_names=tuple(all_names),
                out_names=tuple(out_names),
                lowering_input_output_aliases=(),
                sim_require_finite=True,
                sim_require_nnan=True,
                nc=nc,
            )
            return tuple(outs)

        devices = jax.devices()[:n_cores]
        assert len(devices) >= n_cores or len(devices) == n_cores, \
            f"need {n_cores} cores, have {len(jax.devices())}"
        self.mesh = Mesh(np.asarray(devices), ("core",))
        in_specs = (PartitionSpec("core"),) * (n_params + n_outs)
        out_specs = (PartitionSpec("core"),) * n_outs
        self.fn = jax.jit(
            shard_map(_body, mesh=self.mesh, in_specs=in_specs,
                      out_specs=out_specs, check_rep=False),
            keep_unused=True,
        )

    def run(self, in_maps):
        concat_in = [
            np.concatenate([np.asarray(in_maps[c][nm])
                            for c in range(self.n_cores)], axis=0)
            for nm in self.in_names
        ]
        concat_zeros = [
            np.zeros((self.n_cores * av.shape[0], *av.shape[1:]), av.dtype)
            for av in self.out_avals
        ]
        outs = self.fn(*(concat_in + concat_zeros))
        jax.block_until_ready(outs)
        res = []
        for c in range(self.n_cores):
            d = {}
            for i, nm in enumerate(self.out_names):
                a = np.asarray(outs[i]).reshape(self.n_cores,
                                                *self.out_avals[i].shape)
                d[nm] = a[c]
            res.append(d)
        return res


_CACHE = {}


def _get_programs(edge_index):
    key = hash(np.asarray(edge_index).tobytes())
    if key not in _CACHE:
        pre = _preprocess(edge_index)
        ncM = _build_merged(pre)
        _CACHE[key] = (pre, _SpmdRunner(ncM))
    return _CACHE[key]


def _make_maps(pre, x, W1, b1, W2, b2):
    bb1 = np.tile(b1, (128, 1)).astype(np.float32)
    bb2 = np.tile(b2, (128, 1)).astype(np.float32)
    maps = []
    for c in range(N_CORES):
        lo = c * ROWS
        hi = min((c + 1) * ROWS, N_NODES)
        xs = np.zeros((ROWS, IN_CH), np.float32)
        xs[:hi - lo] = x[lo:hi]
        maps.append({
            "xT": np.ascontiguousarray(xs.T), "w1": W1, "w2": W2,
            "dinvA": pre["dinvA"][c], "dinvS": pre["dinvS"][c],
            "bb1": bb1, "bb2": bb2,
            "idxw1": pre["L1"]["idxw"][c], "dstv1": pre["L1"]["dstv"][c],
            "idxw2": pre["L2"]["idxw"][c], "dstv2": pre["L2"]["dstv"][c],
        })
    return maps


def kernel(x, edge_index, W1, b1, W2, b2):
    x = np.asarray(x, np.float32)
    W1 = np.asarray(W1, np.float32)
    b1 = np.asarray(b1, np.float32)
    W2 = np.asarray(W2, np.float32)
    b2 = np.asarray(b2, np.float32)
    pre, rM = _get_programs(edge_index)
    maps = _make_maps(pre, x, W1, b1, W2, b2)
    res = rM.run(maps)
    tile_of = pre["tile_of"]
    out = np.zeros((N_NODES, OUT_CH), np.float32)
    for c in range(N_CORES):
        o = res[c]["outp"]
        for t in range(TPC):
            tl = tile_of[c, t]
            if tl >= 0:
                out[tl * 128:(tl + 1) * 128] = o[t * 128:(t + 1) * 128]
    return out
